# revision 12
# baseline (speedup 1.0000x reference)
"""Kohonen SOM distance-matrix kernel for Trainium2 (Bass/Tile).

Computes sqrt(||x||^2 + ||w||^2 - 2 x.w) for x [32768, 256] against a codebook
w [2500, 256] -> out [32768, 2500], data-parallel over 8 NeuronCores (batch
sharded, codebook replicated).

Per core (batch shard of 4096 rows, m-tiles of 128):
  - TensorE: fp8(e4m3) DoubleRow matmul (K=256 in one pass) computes
    cross = -2 x.w into PSUM (5x 512-col slices, 3x [128,1024] PSUM tiles).
  - Column ranges per m-tile, split across engines to balance load:
    * F [0:f): a full-rank K=128 f16 matmul (ones.T @ wsq_rows, all PE rows
      active so the HAM clock stays unthrottled) accumulates ||w||^2 into
      PSUM; ScalarE then does out = sqrt(psum + ||x||^2) via its
      per-partition bias, PSUM -> SBUF f16.
    * V [f:f+v): VectorE STT computes d2 = (psum + xsq) + wsq_bc in one op
      (PSUM -> SBUF f32); ScalarE sqrt -> f16.
    * G [f+v:N): same STT d2, then GpSimd evaluates a quadratic minimax fit
      p(y)=c2*(y-r1)*(y-r2) of sqrt in 2 ops -> f16. Fit range estimated
      host-side from sampled rows; coefficients ride in as tensors.
  - Output stored as f16 (halves HBM store traffic), host upcasts to f32.
"""

import json
import os

import numpy as np

N_CORES = 8
BATCH = 32768
BS = BATCH // N_CORES  # 4096 rows per core
N = 2500
D = 256
M_TILE = 128
M_TILES = BS // M_TILE  # 32

# PSUM: 8 banks x 512 f32. Three [128, 1024] tiles per m-tile (last holds 452).
PS_W = 1024
MM_SLICES = [(0, 512), (512, 512), (1024, 512), (1536, 512), (2048, 452)]

DEFAULT_CFG = {
    "f_cols": 512,    # fold + ScalarE direct-sqrt columns (multiple of 512)
    "v_cols": 1476,   # STT-d2 + ScalarE sqrt columns
    "g_cols": 512,    # STT-d2 + GpSimd quadratic columns (f+v+g == N)
    "warm_mm": 26,    # PE warm-up matmuls bridging the input-load phase
    "x_chunks": 4,
    "store_alt": False,  # alternate output stores across SP/ACT queues
    "psum_bufs": 4,
}

_CACHE = {}


def _cfg():
    cfg = dict(DEFAULT_CFG)
    env = os.environ.get("BASS_SOM_CFG")
    if env:
        cfg.update(json.loads(env))
    assert cfg["f_cols"] + cfg["v_cols"] + cfg["g_cols"] == N, cfg
    assert cfg["f_cols"] % 512 == 0, cfg
    return cfg


def _tile_splits(c0, c1):
    """Split global col range [c0, c1) at PSUM-tile boundaries ->
    (tile_idx, local_lo, global_lo, width)."""
    out = []
    c = c0
    while c < c1:
        t = c // PS_W
        hi = min(c1, (t + 1) * PS_W, N)
        out.append((t, c - t * PS_W, c, hi - c))
        c = hi
    return out


def _build_bass(cfg=None):
    import concourse.mybir as mybir
    from concourse import bacc
    from concourse.tile import TileContext

    cfg = cfg or _cfg()
    F, V, G = cfg["f_cols"], cfg["v_cols"], cfg["g_cols"]

    f32 = mybir.dt.float32
    f16 = mybir.dt.float16
    bf16 = mybir.dt.bfloat16
    fp8 = mybir.dt.float8e4
    DR = mybir.MatmulPerfMode.DoubleRow
    ADD = mybir.AluOpType.add
    MULT = mybir.AluOpType.mult
    SQRT = mybir.ActivationFunctionType.Sqrt

    x_chunks = cfg["x_chunks"]
    mc = BS // x_chunks  # m columns per x chunk

    nc = bacc.Bacc("TRN2", target_bir_lowering=False, debug=False)
    xt8_d = nc.dram_tensor("xt8", [128, 2, BS], fp8, kind="ExternalInput")
    wt8_d = nc.dram_tensor("wt8", [128, 2, N], fp8, kind="ExternalInput")
    wfr_d = nc.dram_tensor("wfr", [128, max(F, 1)], f16, kind="ExternalInput")
    wbc_d = nc.dram_tensor("wbc", [128, N], f16, kind="ExternalInput")
    xsqb_d = nc.dram_tensor("xsqb", [M_TILE, M_TILES], f32, kind="ExternalInput")
    xr1_d = nc.dram_tensor("xr1", [M_TILE, M_TILES], f32, kind="ExternalInput")
    xr2_d = nc.dram_tensor("xr2", [M_TILE, M_TILES], f32, kind="ExternalInput")
    nr1_d = nc.dram_tensor("nr1", [M_TILE, 1], f32, kind="ExternalInput")
    nr2_d = nc.dram_tensor("nr2", [M_TILE, 1], f32, kind="ExternalInput")
    qc2_d = nc.dram_tensor("qc2", [M_TILE, 1], f32, kind="ExternalInput")
    out = nc.dram_tensor("out", [BS, N], f16, kind="ExternalOutput")

    with TileContext(nc) as tc:
        with (
            tc.tile_pool(name="wpool", bufs=1) as wpool,
            tc.tile_pool(name="xpool", bufs=1) as xpool,
            tc.tile_pool(name="bpool", bufs=1) as bpool,
            tc.tile_pool(name="opool", bufs=4) as opool,
            tc.tile_pool(name="dpool", bufs=4) as dpool,
            tc.tile_pool(name="gpool", bufs=4) as gpool,
            tc.tile_pool(name="pp", bufs=cfg["psum_bufs"], space="PSUM") as pp,
        ):
            # --- PE warm-up: no DMA deps, issues at t=0 while inputs load
            # (HAM un-throttle 1.2 -> 2.4 GHz needs ~3.4us of activity; bridge
            # until the first real matmul so it doesn't re-throttle).
            warm_src = bpool.tile([M_TILE, 512], bf16)
            nc.vector.memset(warm_src, 0.0)
            warm_ps = pp.tile([M_TILE, PS_W], f32, name="ps")
            for _ in range(cfg["warm_mm"]):
                nc.tensor.matmul(
                    warm_ps[:, :512], lhsT=warm_src[:, :M_TILE], rhs=warm_src,
                    start=True, stop=True,
                )
            # Preload the sqrt ACT table set during the load phase (the
            # implicit ACT_TABLE_LOAD costs ~2.6us at first use otherwise).
            warm_act = bpool.tile([M_TILE, 1], f32)
            nc.scalar.activation(
                warm_act, warm_src[:, 0:1], SQRT, bias=0.0, scale=1.0
            )

            # --- input loads. w + small tensors on the SP queue, x chunks on
            # the ACT queue.
            # Load w in slices matching the matmul column slices so the
            # first matmul only waits on its own slice.
            wt8 = wpool.tile([128, 2, N], fp8)
            for g0, gw in MM_SLICES:
                nc.sync.dma_start(
                    wt8[:, :, g0 : g0 + gw], wt8_d[:, :, g0 : g0 + gw]
                )
            if F:
                wfr = bpool.tile([128, F], f16)
                nc.sync.dma_start(wfr, wfr_d[:, :])
                ones = bpool.tile([128, M_TILE], f16)
                nc.vector.memset(ones, 1.0)
            wbc = bpool.tile([128, N], f16)
            nc.sync.dma_start(wbc, wbc_d[:, :])
            xsqb = bpool.tile([M_TILE, M_TILES], f32)
            nc.sync.dma_start(xsqb, xsqb_d[:, :])
            xr1 = bpool.tile([M_TILE, M_TILES], f32)
            nc.sync.dma_start(xr1, xr1_d[:, :])
            xr2 = bpool.tile([M_TILE, M_TILES], f32)
            nc.sync.dma_start(xr2, xr2_d[:, :])
            nr1 = bpool.tile([M_TILE, 1], f32)
            nc.sync.dma_start(nr1, nr1_d[:, :])
            nr2 = bpool.tile([M_TILE, 1], f32)
            nc.sync.dma_start(nr2, nr2_d[:, :])
            qc2 = bpool.tile([M_TILE, 1], f32)
            nc.sync.dma_start(qc2, qc2_d[:, :])

            x_sb = []
            for ci in range(x_chunks):
                xc = xpool.tile([128, 2, mc], fp8, name=f"x{ci}")
                nc.scalar.dma_start(xc, xt8_d[:, :, ci * mc : (ci + 1) * mc])
                x_sb.append(xc)

            # --- main loop over batch tiles.
            for m in range(M_TILES):
                ms = slice(m * M_TILE, (m + 1) * M_TILE)
                mo = slice((m * M_TILE) % mc, (m * M_TILE) % mc + M_TILE)
                xt = x_sb[(m * M_TILE) // mc]
                mb = slice(m, m + 1)
                ot = opool.tile([M_TILE, N], f16, name="ot")
                ps = [
                    pp.tile([M_TILE, PS_W], f32, name="ps") for _ in range(3)
                ]
                for g0, gw in MM_SLICES:
                    t, l0 = g0 // PS_W, g0 % PS_W
                    dst = ps[t][:, l0 : l0 + gw]
                    fold = g0 < F  # fold slices are 512-aligned
                    nc.tensor.matmul(
                        dst, lhsT=xt[:, :, mo], rhs=wt8[:, :, g0 : g0 + gw],
                        start=True, stop=not fold, perf_mode=DR,
                    )
                    if fold:
                        nc.tensor.matmul(
                            dst, lhsT=ones, rhs=wfr[:, g0 : g0 + gw],
                            start=False, stop=True,
                        )

                # F: fused sqrt(psum + xsq) straight out of PSUM.
                for t, l0, c0, w in _tile_splits(0, F):
                    nc.scalar.activation(
                        ot[:, c0 : c0 + w], ps[t][:, l0 : l0 + w], SQRT,
                        bias=xsqb[:, mb], scale=1.0,
                    )

                # V+G: d2 = (psum + xsq) + wsq_bc on VectorE, into one SBUF
                # tile (ScalarE and GpSimd then read disjoint slices of it).
                d2 = dpool.tile([M_TILE, V + G], f32, name="d2")
                for t, l0, c0, w in _tile_splits(F, N):
                    nc.vector.scalar_tensor_tensor(
                        d2[:, c0 - F : c0 - F + w], ps[t][:, l0 : l0 + w],
                        xsqb[:, mb], wbc[:, c0 : c0 + w], ADD, ADD,
                    )
                if V:
                    nc.scalar.activation(
                        ot[:, F : F + V], d2[:, :V], SQRT, bias=0.0, scale=1.0
                    )
                if G:
                    # Pool rejects STT; 3-op variant: (d2-r1)*c2, (d2-r2), mul
                    t1g = gpool.tile([M_TILE, G], f32, name="t1g")
                    nc.gpsimd.tensor_scalar(
                        t1g, d2[:, V:], nr1[:, 0:1], qc2[:, 0:1], ADD, MULT
                    )
                    t2g = gpool.tile([M_TILE, G], f32, name="t2g")
                    nc.gpsimd.tensor_scalar(
                        t2g, d2[:, V:], nr2[:, 0:1], None, ADD
                    )
                    nc.gpsimd.tensor_tensor(
                        ot[:, F + V :], t1g, t2g, MULT
                    )

                # All stores on the SP queue: the Scalar engine is near its
                # busy ceiling and doorbell rings there are not free.
                eng = nc.scalar if (m % 2 == 1 and cfg["store_alt"]) else nc.sync
                eng.dma_start(out[ms, :], ot)

    nc.finalize()
    return nc


def _quad_fit(x, w, xsq, wsq):
    """Sampled-range quadratic minimax fit of sqrt on the d2 range.

    Returns (c2, r1, r2) with sqrt(y) ~= c2*(y-r1)*(y-r2) on the range."""
    rng = np.random.default_rng(12345)
    rows = rng.choice(x.shape[0], 768, replace=False)
    cross = x[rows].astype(np.float32) @ (-2.0 * w.astype(np.float32)).T
    d2 = cross + wsq[None, :].astype(np.float32) + xsq[rows, None].astype(
        np.float32
    )
    smin, smax = float(d2.min()), float(d2.max())
    span = smax - smin
    lo, hi = max(smin - 0.12 * span, 1e-3), smax + 0.12 * span
    yy = np.polynomial.chebyshev.chebpts1(512) * (hi - lo) / 2 + (lo + hi) / 2
    cf = np.polyfit(yy, np.sqrt(yy), 2, w=1.0 / np.sqrt(yy))
    roots = np.roots(cf)
    assert np.isreal(roots).all(), (cf, roots)
    r1, r2 = sorted(roots.real)
    return float(cf[0]), float(r1), float(r2)


def _prep_inputs(x, weights):
    import ml_dtypes

    cfg = _cfg()
    F = cfg["f_cols"]
    x = np.ascontiguousarray(np.asarray(x, dtype=np.float32))
    w = np.ascontiguousarray(np.asarray(weights, dtype=np.float32))
    assert x.shape == (BATCH, D), x.shape
    assert w.shape == (N, D), w.shape

    xsq = np.einsum("bd,bd->b", x, x)
    wsq = np.einsum("nd,nd->n", w, w)
    c2, r1, r2 = _quad_fit(x, w, xsq, wsq)

    fp8 = ml_dtypes.float8_e4m3
    xq = x.astype(fp8)  # [B, 256]
    wq = (-2.0 * w).astype(fp8)  # [N, 256]
    # DoubleRow packing: [p, t, cols] with contraction row = 128*t + p.
    wt8 = np.ascontiguousarray(wq.reshape(N, 2, 128).transpose(2, 1, 0))

    # Full-rank fold operand: 128 f16 rows summing to wsq (residual row last).
    wfr = np.tile((wsq[:max(F, 1)] / 128).astype(np.float16), (128, 1))
    resid = wsq[:max(F, 1)] - wfr.astype(np.float32).sum(axis=0)
    wfr[127] = (wfr[127].astype(np.float32) + resid).astype(np.float16)
    wbc = np.tile(wsq.astype(np.float16), (128, 1))  # [128, N] broadcast

    qc2 = np.full((M_TILE, 1), c2, np.float32)
    nr1 = np.full((M_TILE, 1), -r1, np.float32)
    nr2 = np.full((M_TILE, 1), -r2, np.float32)

    in_maps = []
    for c in range(N_CORES):
        bs = slice(c * BS, (c + 1) * BS)
        xt8 = np.ascontiguousarray(
            xq[bs].reshape(BS, 2, 128).transpose(2, 1, 0)
        )  # [128, 2, BS]
        xsq_t = np.ascontiguousarray(
            xsq[bs].reshape(M_TILES, M_TILE).T
        )  # [128, 32]
        in_maps.append(
            {
                "xt8": xt8,
                "wt8": wt8,
                "wfr": np.ascontiguousarray(wfr),
                "wbc": np.ascontiguousarray(wbc),
                "xsqb": xsq_t,
                "xr1": np.ascontiguousarray(xsq_t - np.float32(r1)),
                "xr2": np.ascontiguousarray(xsq_t - np.float32(r2)),
                "nr1": nr1,
                "nr2": nr2,
                "qc2": qc2,
            }
        )
    return in_maps


def run(x, weights, trace=False, nc=None, **kwargs):
    from concourse.bass_utils import run_bass_kernel_spmd

    if nc is None:
        if "nc" not in _CACHE:
            _CACHE["nc"] = _build_bass()
        nc = _CACHE["nc"]
    in_maps = _prep_inputs(x, weights)
    res = run_bass_kernel_spmd(
        nc, in_maps, core_ids=list(range(N_CORES)), trace=trace, **kwargs
    )
    out = np.concatenate(
        [res.results[c]["out"].astype(np.float32) for c in range(N_CORES)],
        axis=0,
    )
    return out, res


def _get_runner():
    """Build + jit the SPMD executable once; reuse across kernel() calls."""
    if "runner" in _CACHE:
        return _CACHE["runner"]

    import jax
    import concourse.mybir as mybir
    from concourse import bass2jax
    from jax.sharding import Mesh, PartitionSpec
    from jax.experimental.shard_map import shard_map

    bass2jax.install_neuronx_cc_hook()
    if "nc" not in _CACHE:
        _CACHE["nc"] = _build_bass()
    nc = _CACHE["nc"]

    partition_name = (
        nc.partition_id_tensor.name if nc.partition_id_tensor else None
    )
    in_names, out_names, out_avals, zero_templates = [], [], [], []
    for alloc in nc.m.functions[0].allocations:
        if not isinstance(alloc, mybir.MemoryLocationSet):
            continue
        name = alloc.memorylocations[0].name
        if alloc.kind == "ExternalInput":
            if name != partition_name:
                in_names.append(name)
        elif alloc.kind == "ExternalOutput":
            out_names.append(name)
            shape = tuple(alloc.tensor_shape)
            dtype = mybir.dt.np(alloc.dtype)
            out_avals.append(jax.core.ShapedArray(shape, dtype))
            zero_templates.append((shape, dtype))
    n_params = len(in_names)
    n_outs = len(out_names)
    all_names = in_names + out_names
    if partition_name is not None:
        all_names = all_names + [partition_name]
    donate = tuple(range(n_params, n_params + n_outs))

    def _body(*args):
        operands = list(args)
        if partition_name is not None:
            operands.append(bass2jax.partition_id_tensor())
        outs = bass2jax._bass_exec_p.bind(
            *operands,
            out_avals=tuple(out_avals),
            in_names=tuple(all_names),
            out_names=tuple(out_names),
            lowering_input_output_aliases=(),
            sim_require_finite=True,
            sim_require_nnan=True,
            nc=nc,
        )
        return tuple(outs)

    devices = jax.devices()[:N_CORES]
    mesh = Mesh(np.asarray(devices), ("core",))
    specs = (PartitionSpec("core"),) * (n_params + n_outs)
    sharded = jax.jit(
        shard_map(
            _body, mesh=mesh, in_specs=specs, out_specs=specs[:n_outs],
            check_rep=False,
        ),
        donate_argnums=donate,
        keep_unused=True,
    )

    def runner(in_maps):
        concat_in = [
            np.concatenate([m[name] for m in in_maps], axis=0)
            for name in in_names
        ]
        concat_zeros = [
            np.zeros((N_CORES * s[0], *s[1:]), d) for s, d in zero_templates
        ]
        out_arrs = sharded(*concat_in, *concat_zeros)
        return np.asarray(out_arrs[out_names.index("out")])

    _CACHE["runner"] = runner
    return runner


def kernel(x, weights):
    runner = _get_runner()
    in_maps = _prep_inputs(x, weights)
    out = runner(in_maps)
    return out.astype(np.float32)


# revision 13
# speedup vs baseline: 3.0810x; 3.0810x over previous
"""Kohonen SOM distance-matrix kernel for Trainium2 (Bass/Tile).

Computes sqrt(||x||^2 + ||w||^2 - 2 x.w) for x [32768, 256] against a codebook
w [2500, 256] -> out [32768, 2500], data-parallel over 8 NeuronCores (batch
sharded, codebook replicated).

Per core (batch shard of 4096 rows, m-tiles of 128):
  - TensorE: fp8(e4m3) DoubleRow matmul (K=256 in one pass) computes
    cross = -2 x.w into PSUM (5x 512-col slices, 3x [128,1024] PSUM tiles).
  - Column ranges per m-tile, split across engines to balance load:
    * F [0:f): a full-rank K=128 f16 matmul (ones.T @ wsq_rows, all PE rows
      active so the HAM clock stays unthrottled) accumulates ||w||^2 into
      PSUM; ScalarE then does out = sqrt(psum + ||x||^2) via its
      per-partition bias, PSUM -> SBUF f16.
    * V [f:f+v): VectorE STT computes d2 = (psum + xsq) + wsq_bc in one op
      (PSUM -> SBUF f32); ScalarE sqrt -> f16.
    * G [f+v:N): same STT d2, then GpSimd evaluates a quadratic minimax fit
      p(y)=c2*(y-r1)*(y-r2) of sqrt in 2 ops -> f16. Fit range estimated
      host-side from sampled rows; coefficients ride in as tensors.
  - Output stored as f16 (halves HBM store traffic), host upcasts to f32.
"""

import json
import os

import numpy as np

N_CORES = 8
BATCH = 32768
BS = BATCH // N_CORES  # 4096 rows per core
N = 2500
D = 256
M_TILE = 128
M_TILES = BS // M_TILE  # 32

# PSUM: 8 banks x 512 f32. Three [128, 1024] tiles per m-tile (last holds 452).
PS_W = 1024
MM_SLICES = [(0, 512), (512, 512), (1024, 512), (1536, 512), (2048, 452)]

DEFAULT_CFG = {
    "f_cols": 512,    # fold + ScalarE direct-sqrt columns (multiple of 512)
    "v_cols": 1988,   # STT-d2 + ScalarE sqrt columns
    "g_cols": 0,      # STT-d2 + GpSimd quad columns (Pool TS is ~8x too slow)
    "warm_mm": 26,    # PE warm-up matmuls bridging the input-load phase
    "x_chunks": 4,
    "store_alt": False,  # alternate output stores across SP/ACT queues
    "psum_bufs": 4,
}

_CACHE = {}


def _cfg():
    cfg = dict(DEFAULT_CFG)
    env = os.environ.get("BASS_SOM_CFG")
    if env:
        cfg.update(json.loads(env))
    assert cfg["f_cols"] + cfg["v_cols"] + cfg["g_cols"] == N, cfg
    assert cfg["f_cols"] % 512 == 0, cfg
    return cfg


def _tile_splits(c0, c1):
    """Split global col range [c0, c1) at PSUM-tile boundaries ->
    (tile_idx, local_lo, global_lo, width)."""
    out = []
    c = c0
    while c < c1:
        t = c // PS_W
        hi = min(c1, (t + 1) * PS_W, N)
        out.append((t, c - t * PS_W, c, hi - c))
        c = hi
    return out


def _build_bass(cfg=None):
    import concourse.mybir as mybir
    from concourse import bacc
    from concourse.tile import TileContext

    cfg = cfg or _cfg()
    F, V, G = cfg["f_cols"], cfg["v_cols"], cfg["g_cols"]

    f32 = mybir.dt.float32
    f16 = mybir.dt.float16
    bf16 = mybir.dt.bfloat16
    fp8 = mybir.dt.float8e4
    DR = mybir.MatmulPerfMode.DoubleRow
    ADD = mybir.AluOpType.add
    MULT = mybir.AluOpType.mult
    SQRT = mybir.ActivationFunctionType.Sqrt

    x_chunks = cfg["x_chunks"]
    mc = BS // x_chunks  # m columns per x chunk

    nc = bacc.Bacc("TRN2", target_bir_lowering=False, debug=False)
    xt8_d = nc.dram_tensor("xt8", [128, 2, BS], fp8, kind="ExternalInput")
    wt8_d = nc.dram_tensor("wt8", [128, 2, N], fp8, kind="ExternalInput")
    wfr_d = nc.dram_tensor("wfr", [128, max(F, 1)], f16, kind="ExternalInput")
    wbc_d = nc.dram_tensor("wbc", [128, N], f16, kind="ExternalInput")
    xsqb_d = nc.dram_tensor("xsqb", [M_TILE, M_TILES], f32, kind="ExternalInput")
    xr1_d = nc.dram_tensor("xr1", [M_TILE, M_TILES], f32, kind="ExternalInput")
    xr2_d = nc.dram_tensor("xr2", [M_TILE, M_TILES], f32, kind="ExternalInput")
    nr1_d = nc.dram_tensor("nr1", [M_TILE, 1], f32, kind="ExternalInput")
    nr2_d = nc.dram_tensor("nr2", [M_TILE, 1], f32, kind="ExternalInput")
    qc2_d = nc.dram_tensor("qc2", [M_TILE, 1], f32, kind="ExternalInput")
    out = nc.dram_tensor("out", [BS, N], f16, kind="ExternalOutput")

    with TileContext(nc) as tc:
        with (
            tc.tile_pool(name="wpool", bufs=1) as wpool,
            tc.tile_pool(name="xpool", bufs=1) as xpool,
            tc.tile_pool(name="bpool", bufs=1) as bpool,
            tc.tile_pool(name="opool", bufs=4) as opool,
            tc.tile_pool(name="dpool", bufs=4) as dpool,
            tc.tile_pool(name="gpool", bufs=4) as gpool,
            tc.tile_pool(name="pp", bufs=cfg["psum_bufs"], space="PSUM") as pp,
        ):
            # --- PE warm-up: no DMA deps, issues at t=0 while inputs load
            # (HAM un-throttle 1.2 -> 2.4 GHz needs ~3.4us of activity; bridge
            # until the first real matmul so it doesn't re-throttle).
            warm_src = bpool.tile([M_TILE, 512], bf16)
            nc.vector.memset(warm_src, 0.0)
            warm_ps = pp.tile([M_TILE, PS_W], f32, name="ps")
            for _ in range(cfg["warm_mm"]):
                nc.tensor.matmul(
                    warm_ps[:, :512], lhsT=warm_src[:, :M_TILE], rhs=warm_src,
                    start=True, stop=True,
                )
            # Preload the sqrt ACT table set during the load phase (the
            # implicit ACT_TABLE_LOAD costs ~2.6us at first use otherwise).
            warm_act = bpool.tile([M_TILE, 1], f32)
            nc.scalar.activation(
                warm_act, warm_src[:, 0:1], SQRT, bias=0.0, scale=1.0
            )

            # --- input loads. w + small tensors on the SP queue, x chunks on
            # the ACT queue.
            # Load w in slices matching the matmul column slices so the
            # first matmul only waits on its own slice.
            wt8 = wpool.tile([128, 2, N], fp8)
            for g0, gw in MM_SLICES:
                nc.sync.dma_start(
                    wt8[:, :, g0 : g0 + gw], wt8_d[:, :, g0 : g0 + gw]
                )
            if F:
                wfr = bpool.tile([128, F], f16)
                nc.sync.dma_start(wfr, wfr_d[:, :])
                ones = bpool.tile([128, M_TILE], f16)
                nc.vector.memset(ones, 1.0)
            wbc = bpool.tile([128, N], f16)
            nc.sync.dma_start(wbc, wbc_d[:, :])
            xsqb = bpool.tile([M_TILE, M_TILES], f32)
            nc.sync.dma_start(xsqb, xsqb_d[:, :])
            xr1 = bpool.tile([M_TILE, M_TILES], f32)
            nc.sync.dma_start(xr1, xr1_d[:, :])
            xr2 = bpool.tile([M_TILE, M_TILES], f32)
            nc.sync.dma_start(xr2, xr2_d[:, :])
            nr1 = bpool.tile([M_TILE, 1], f32)
            nc.sync.dma_start(nr1, nr1_d[:, :])
            nr2 = bpool.tile([M_TILE, 1], f32)
            nc.sync.dma_start(nr2, nr2_d[:, :])
            qc2 = bpool.tile([M_TILE, 1], f32)
            nc.sync.dma_start(qc2, qc2_d[:, :])

            x_sb = []
            for ci in range(x_chunks):
                xc = xpool.tile([128, 2, mc], fp8, name=f"x{ci}")
                nc.scalar.dma_start(xc, xt8_d[:, :, ci * mc : (ci + 1) * mc])
                x_sb.append(xc)

            # --- main loop over batch tiles.
            for m in range(M_TILES):
                ms = slice(m * M_TILE, (m + 1) * M_TILE)
                mo = slice((m * M_TILE) % mc, (m * M_TILE) % mc + M_TILE)
                xt = x_sb[(m * M_TILE) // mc]
                mb = slice(m, m + 1)
                ot = opool.tile([M_TILE, N], f16, name="ot")
                ps = [
                    pp.tile([M_TILE, PS_W], f32, name="ps") for _ in range(3)
                ]
                for g0, gw in MM_SLICES:
                    t, l0 = g0 // PS_W, g0 % PS_W
                    dst = ps[t][:, l0 : l0 + gw]
                    fold = g0 < F  # fold slices are 512-aligned
                    nc.tensor.matmul(
                        dst, lhsT=xt[:, :, mo], rhs=wt8[:, :, g0 : g0 + gw],
                        start=True, stop=not fold, perf_mode=DR,
                    )
                    if fold:
                        nc.tensor.matmul(
                            dst, lhsT=ones, rhs=wfr[:, g0 : g0 + gw],
                            start=False, stop=True,
                        )

                # F: fused sqrt(psum + xsq) straight out of PSUM.
                for t, l0, c0, w in _tile_splits(0, F):
                    nc.scalar.activation(
                        ot[:, c0 : c0 + w], ps[t][:, l0 : l0 + w], SQRT,
                        bias=xsqb[:, mb], scale=1.0,
                    )

                # V+G: d2 = (psum + xsq) + wsq_bc on VectorE, into one SBUF
                # tile (ScalarE and GpSimd then read disjoint slices of it).
                d2 = dpool.tile([M_TILE, V + G], f32, name="d2")
                for t, l0, c0, w in _tile_splits(F, N):
                    nc.vector.scalar_tensor_tensor(
                        d2[:, c0 - F : c0 - F + w], ps[t][:, l0 : l0 + w],
                        xsqb[:, mb], wbc[:, c0 : c0 + w], ADD, ADD,
                    )
                if V:
                    nc.scalar.activation(
                        ot[:, F : F + V], d2[:, :V], SQRT, bias=0.0, scale=1.0
                    )
                if G:
                    # Pool rejects STT; 3-op variant: (d2-r1)*c2, (d2-r2), mul
                    t1g = gpool.tile([M_TILE, G], f32, name="t1g")
                    nc.gpsimd.tensor_scalar(
                        t1g, d2[:, V:], nr1[:, 0:1], qc2[:, 0:1], ADD, MULT
                    )
                    t2g = gpool.tile([M_TILE, G], f32, name="t2g")
                    nc.gpsimd.tensor_scalar(
                        t2g, d2[:, V:], nr2[:, 0:1], None, ADD
                    )
                    nc.gpsimd.tensor_tensor(
                        ot[:, F + V :], t1g, t2g, MULT
                    )

                # All stores on the SP queue: the Scalar engine is near its
                # busy ceiling and doorbell rings there are not free.
                eng = nc.scalar if (m % 2 == 1 and cfg["store_alt"]) else nc.sync
                eng.dma_start(out[ms, :], ot)

    nc.finalize()
    return nc


def _quad_fit(x, w, xsq, wsq):
    """Sampled-range quadratic minimax fit of sqrt on the d2 range.

    Returns (c2, r1, r2) with sqrt(y) ~= c2*(y-r1)*(y-r2) on the range."""
    rng = np.random.default_rng(12345)
    rows = rng.choice(x.shape[0], 768, replace=False)
    cross = x[rows].astype(np.float32) @ (-2.0 * w.astype(np.float32)).T
    d2 = cross + wsq[None, :].astype(np.float32) + xsq[rows, None].astype(
        np.float32
    )
    smin, smax = float(d2.min()), float(d2.max())
    span = smax - smin
    lo, hi = max(smin - 0.12 * span, 1e-3), smax + 0.12 * span
    yy = np.polynomial.chebyshev.chebpts1(512) * (hi - lo) / 2 + (lo + hi) / 2
    cf = np.polyfit(yy, np.sqrt(yy), 2, w=1.0 / np.sqrt(yy))
    roots = np.roots(cf)
    assert np.isreal(roots).all(), (cf, roots)
    r1, r2 = sorted(roots.real)
    return float(cf[0]), float(r1), float(r2)


def _prep_inputs(x, weights):
    import ml_dtypes

    cfg = _cfg()
    F = cfg["f_cols"]
    x = np.ascontiguousarray(np.asarray(x, dtype=np.float32))
    w = np.ascontiguousarray(np.asarray(weights, dtype=np.float32))
    assert x.shape == (BATCH, D), x.shape
    assert w.shape == (N, D), w.shape

    xsq = np.einsum("bd,bd->b", x, x)
    wsq = np.einsum("nd,nd->n", w, w)
    c2, r1, r2 = _quad_fit(x, w, xsq, wsq)

    fp8 = ml_dtypes.float8_e4m3
    xq = x.astype(fp8)  # [B, 256]
    wq = (-2.0 * w).astype(fp8)  # [N, 256]
    # DoubleRow packing: [p, t, cols] with contraction row = 128*t + p.
    wt8 = np.ascontiguousarray(wq.reshape(N, 2, 128).transpose(2, 1, 0))

    # Full-rank fold operand: 128 f16 rows summing to wsq (residual row last).
    wfr = np.tile((wsq[:max(F, 1)] / 128).astype(np.float16), (128, 1))
    resid = wsq[:max(F, 1)] - wfr.astype(np.float32).sum(axis=0)
    wfr[127] = (wfr[127].astype(np.float32) + resid).astype(np.float16)
    wbc = np.tile(wsq.astype(np.float16), (128, 1))  # [128, N] broadcast

    qc2 = np.full((M_TILE, 1), c2, np.float32)
    nr1 = np.full((M_TILE, 1), -r1, np.float32)
    nr2 = np.full((M_TILE, 1), -r2, np.float32)

    in_maps = []
    for c in range(N_CORES):
        bs = slice(c * BS, (c + 1) * BS)
        xt8 = np.ascontiguousarray(
            xq[bs].reshape(BS, 2, 128).transpose(2, 1, 0)
        )  # [128, 2, BS]
        xsq_t = np.ascontiguousarray(
            xsq[bs].reshape(M_TILES, M_TILE).T
        )  # [128, 32]
        in_maps.append(
            {
                "xt8": xt8,
                "wt8": wt8,
                "wfr": np.ascontiguousarray(wfr),
                "wbc": np.ascontiguousarray(wbc),
                "xsqb": xsq_t,
                "xr1": np.ascontiguousarray(xsq_t - np.float32(r1)),
                "xr2": np.ascontiguousarray(xsq_t - np.float32(r2)),
                "nr1": nr1,
                "nr2": nr2,
                "qc2": qc2,
            }
        )
    return in_maps


def run(x, weights, trace=False, nc=None, **kwargs):
    from concourse.bass_utils import run_bass_kernel_spmd

    if nc is None:
        if "nc" not in _CACHE:
            _CACHE["nc"] = _build_bass()
        nc = _CACHE["nc"]
    in_maps = _prep_inputs(x, weights)
    res = run_bass_kernel_spmd(
        nc, in_maps, core_ids=list(range(N_CORES)), trace=trace, **kwargs
    )
    out = np.concatenate(
        [res.results[c]["out"].astype(np.float32) for c in range(N_CORES)],
        axis=0,
    )
    return out, res


def _get_runner():
    """Build + jit the SPMD executable once; reuse across kernel() calls."""
    if "runner" in _CACHE:
        return _CACHE["runner"]

    import jax
    import concourse.mybir as mybir
    from concourse import bass2jax
    from jax.sharding import Mesh, PartitionSpec
    from jax.experimental.shard_map import shard_map

    bass2jax.install_neuronx_cc_hook()
    if "nc" not in _CACHE:
        _CACHE["nc"] = _build_bass()
    nc = _CACHE["nc"]

    partition_name = (
        nc.partition_id_tensor.name if nc.partition_id_tensor else None
    )
    in_names, out_names, out_avals, zero_templates = [], [], [], []
    for alloc in nc.m.functions[0].allocations:
        if not isinstance(alloc, mybir.MemoryLocationSet):
            continue
        name = alloc.memorylocations[0].name
        if alloc.kind == "ExternalInput":
            if name != partition_name:
                in_names.append(name)
        elif alloc.kind == "ExternalOutput":
            out_names.append(name)
            shape = tuple(alloc.tensor_shape)
            dtype = mybir.dt.np(alloc.dtype)
            out_avals.append(jax.core.ShapedArray(shape, dtype))
            zero_templates.append((shape, dtype))
    n_params = len(in_names)
    n_outs = len(out_names)
    all_names = in_names + out_names
    if partition_name is not None:
        all_names = all_names + [partition_name]
    donate = tuple(range(n_params, n_params + n_outs))

    def _body(*args):
        operands = list(args)
        if partition_name is not None:
            operands.append(bass2jax.partition_id_tensor())
        outs = bass2jax._bass_exec_p.bind(
            *operands,
            out_avals=tuple(out_avals),
            in_names=tuple(all_names),
            out_names=tuple(out_names),
            lowering_input_output_aliases=(),
            sim_require_finite=True,
            sim_require_nnan=True,
            nc=nc,
        )
        return tuple(outs)

    devices = jax.devices()[:N_CORES]
    mesh = Mesh(np.asarray(devices), ("core",))
    specs = (PartitionSpec("core"),) * (n_params + n_outs)
    sharded = jax.jit(
        shard_map(
            _body, mesh=mesh, in_specs=specs, out_specs=specs[:n_outs],
            check_rep=False,
        ),
        donate_argnums=donate,
        keep_unused=True,
    )

    def runner(in_maps):
        concat_in = [
            np.concatenate([m[name] for m in in_maps], axis=0)
            for name in in_names
        ]
        concat_zeros = [
            np.zeros((N_CORES * s[0], *s[1:]), d) for s, d in zero_templates
        ]
        out_arrs = sharded(*concat_in, *concat_zeros)
        return np.asarray(out_arrs[out_names.index("out")])

    _CACHE["runner"] = runner
    return runner


def kernel(x, weights):
    runner = _get_runner()
    in_maps = _prep_inputs(x, weights)
    out = runner(in_maps)
    return out.astype(np.float32)


# revision 17
# speedup vs baseline: 3.0965x; 1.0051x over previous
"""Kohonen SOM distance-matrix kernel for Trainium2 (Bass/Tile).

Computes sqrt(||x||^2 + ||w||^2 - 2 x.w) for x [32768, 256] against a codebook
w [2500, 256] -> out [32768, 2500], data-parallel over 8 NeuronCores (batch
sharded, codebook replicated).

Per core (batch shard of 4096 rows, m-tiles of 128):
  - TensorE: fp8(e4m3) DoubleRow matmul (K=256 in one pass) computes
    cross = -2 x.w into PSUM (5x 512-col slices, 3x [128,1024] PSUM tiles).
  - Column ranges per m-tile, split across engines to balance load:
    * F [0:f): a full-rank K=128 f16 matmul (ones.T @ wsq_rows, all PE rows
      active so the HAM clock stays unthrottled) accumulates ||w||^2 into
      PSUM; ScalarE then does out = sqrt(psum + ||x||^2) via its
      per-partition bias, PSUM -> SBUF f16.
    * V [f:f+v): VectorE STT computes d2 = (psum + xsq) + wsq_bc in one op
      (PSUM -> SBUF f32); ScalarE sqrt -> f16.
    * G [f+v:N): same STT d2, then GpSimd evaluates a quadratic minimax fit
      p(y)=c2*(y-r1)*(y-r2) of sqrt in 2 ops -> f16. Fit range estimated
      host-side from sampled rows; coefficients ride in as tensors.
  - Output stored as f16 (halves HBM store traffic), host upcasts to f32.
"""

import json
import os

import numpy as np

N_CORES = 8
BATCH = 32768
BS = BATCH // N_CORES  # 4096 rows per core
N = 2500
D = 256
M_TILE = 128
M_TILES = BS // M_TILE  # 32

# PSUM: 8 banks x 512 f32. Three [128, 1024] tiles per m-tile (last holds 452).
PS_W = 1024
MM_SLICES = [(0, 512), (512, 512), (1024, 512), (1536, 512), (2048, 452)]

DEFAULT_CFG = {
    "f_cols": 452,    # fold + ScalarE direct-sqrt columns (tail MM slices)
    "v_cols": 2048,   # STT-d2 + ScalarE sqrt columns (leading, 1024-aligned)
    "g_cols": 0,      # STT-d2 + GpSimd quad columns (Pool TS is ~8x too slow)
    "warm_mm": 20,    # PE warm-up matmuls bridging the input-load phase
    "x_chunks": 8,
    "store_swdge": True,  # alternate stores across SP HWDGE and Pool SWDGE
    "psum_bufs": 4,
}

_CACHE = {}


def _cfg():
    cfg = dict(DEFAULT_CFG)
    env = os.environ.get("BASS_SOM_CFG")
    if env:
        cfg.update(json.loads(env))
    assert cfg["f_cols"] + cfg["v_cols"] + cfg["g_cols"] == N, cfg
    # fold cols sit at the tail and must cover whole trailing MM slices
    lo = N - cfg["f_cols"]
    assert any(lo == g0 for g0, _ in MM_SLICES) or cfg["f_cols"] == 0, cfg
    return cfg


def _tile_splits(c0, c1):
    """Split global col range [c0, c1) at PSUM-tile boundaries ->
    (tile_idx, local_lo, global_lo, width)."""
    out = []
    c = c0
    while c < c1:
        t = c // PS_W
        hi = min(c1, (t + 1) * PS_W, N)
        out.append((t, c - t * PS_W, c, hi - c))
        c = hi
    return out


def _build_bass(cfg=None):
    import concourse.mybir as mybir
    from concourse import bacc
    from concourse.tile import TileContext

    cfg = cfg or _cfg()
    F, V, G = cfg["f_cols"], cfg["v_cols"], cfg["g_cols"]

    f32 = mybir.dt.float32
    f16 = mybir.dt.float16
    bf16 = mybir.dt.bfloat16
    fp8 = mybir.dt.float8e4
    DR = mybir.MatmulPerfMode.DoubleRow
    ADD = mybir.AluOpType.add
    MULT = mybir.AluOpType.mult
    SQRT = mybir.ActivationFunctionType.Sqrt

    x_chunks = cfg["x_chunks"]
    mc = BS // x_chunks  # m columns per x chunk

    nc = bacc.Bacc("TRN2", target_bir_lowering=False, debug=False)
    xt8_d = nc.dram_tensor("xt8", [128, 2, BS], fp8, kind="ExternalInput")
    wt8_d = nc.dram_tensor("wt8", [128, 2, N], fp8, kind="ExternalInput")
    wfr_d = nc.dram_tensor("wfr", [128, max(F, 1)], f16, kind="ExternalInput")
    wbc_d = nc.dram_tensor("wbc", [128, N], f16, kind="ExternalInput")
    xsqb_d = nc.dram_tensor("xsqb", [M_TILE, M_TILES], f32, kind="ExternalInput")
    xr1_d = nc.dram_tensor("xr1", [M_TILE, M_TILES], f32, kind="ExternalInput")
    xr2_d = nc.dram_tensor("xr2", [M_TILE, M_TILES], f32, kind="ExternalInput")
    nr1_d = nc.dram_tensor("nr1", [M_TILE, 1], f32, kind="ExternalInput")
    nr2_d = nc.dram_tensor("nr2", [M_TILE, 1], f32, kind="ExternalInput")
    qc2_d = nc.dram_tensor("qc2", [M_TILE, 1], f32, kind="ExternalInput")
    out = nc.dram_tensor("out", [BS, N], f16, kind="ExternalOutput")

    with TileContext(nc) as tc:
        with (
            tc.tile_pool(name="wpool", bufs=1) as wpool,
            tc.tile_pool(name="xpool", bufs=1) as xpool,
            tc.tile_pool(name="bpool", bufs=1) as bpool,
            tc.tile_pool(name="opool", bufs=4) as opool,
            tc.tile_pool(name="dpool", bufs=4) as dpool,
            tc.tile_pool(name="gpool", bufs=4) as gpool,
            tc.tile_pool(name="pp", bufs=cfg["psum_bufs"], space="PSUM") as pp,
        ):
            # --- PE warm-up: no DMA deps, issues at t=0 while inputs load
            # (HAM un-throttle 1.2 -> 2.4 GHz needs ~3.4us of activity; bridge
            # until the first real matmul so it doesn't re-throttle).
            warm_src = bpool.tile([M_TILE, 512], bf16)
            nc.vector.memset(warm_src, 0.0)
            warm_ps = pp.tile([M_TILE, PS_W], f32, name="ps")
            for _ in range(cfg["warm_mm"]):
                nc.tensor.matmul(
                    warm_ps[:, :512], lhsT=warm_src[:, :M_TILE], rhs=warm_src,
                    start=True, stop=True,
                )
            # Preload the sqrt ACT table set during the load phase (the
            # implicit ACT_TABLE_LOAD costs ~2.6us at first use otherwise).
            warm_act = bpool.tile([M_TILE, 1], f32)
            nc.scalar.activation(
                warm_act, warm_src[:, 0:1], SQRT, bias=0.0, scale=1.0
            )

            # --- input loads. w + small tensors on the SP queue, x chunks on
            # the ACT queue.
            # Load w in slices matching the matmul column slices so the
            # first matmul only waits on its own slice.
            wt8 = wpool.tile([128, 2, N], fp8)
            for g0, gw in MM_SLICES:
                nc.sync.dma_start(
                    wt8[:, :, g0 : g0 + gw], wt8_d[:, :, g0 : g0 + gw]
                )
            if F:
                wfr = bpool.tile([128, F], f16)
                nc.sync.dma_start(wfr, wfr_d[:, :])
                ones = bpool.tile([128, M_TILE], f16)
                nc.vector.memset(ones, 1.0)
            wbc = bpool.tile([128, N], f16)
            nc.sync.dma_start(wbc, wbc_d[:, :])
            xsqb = bpool.tile([M_TILE, M_TILES], f32)
            nc.sync.dma_start(xsqb, xsqb_d[:, :])
            xr1 = bpool.tile([M_TILE, M_TILES], f32)
            nc.sync.dma_start(xr1, xr1_d[:, :])
            xr2 = bpool.tile([M_TILE, M_TILES], f32)
            nc.sync.dma_start(xr2, xr2_d[:, :])
            nr1 = bpool.tile([M_TILE, 1], f32)
            nc.sync.dma_start(nr1, nr1_d[:, :])
            nr2 = bpool.tile([M_TILE, 1], f32)
            nc.sync.dma_start(nr2, nr2_d[:, :])
            qc2 = bpool.tile([M_TILE, 1], f32)
            nc.sync.dma_start(qc2, qc2_d[:, :])

            x_sb = []
            for ci in range(x_chunks):
                xc = xpool.tile([128, 2, mc], fp8, name=f"x{ci}")
                nc.scalar.dma_start(xc, xt8_d[:, :, ci * mc : (ci + 1) * mc])
                x_sb.append(xc)

            # --- main loop over batch tiles.
            for m in range(M_TILES):
                ms = slice(m * M_TILE, (m + 1) * M_TILE)
                mo = slice((m * M_TILE) % mc, (m * M_TILE) % mc + M_TILE)
                xt = x_sb[(m * M_TILE) // mc]
                mb = slice(m, m + 1)
                ot = opool.tile([M_TILE, N], f16, name="ot")
                ps = [
                    pp.tile([M_TILE, PS_W], f32, name="ps") for _ in range(3)
                ]
                F_LO = N - F
                for g0, gw in MM_SLICES:
                    t, l0 = g0 // PS_W, g0 % PS_W
                    dst = ps[t][:, l0 : l0 + gw]
                    fold = g0 >= F_LO  # fold slices sit at the tail
                    nc.tensor.matmul(
                        dst, lhsT=xt[:, :, mo], rhs=wt8[:, :, g0 : g0 + gw],
                        start=True, stop=not fold, perf_mode=DR,
                    )
                    if fold:
                        nc.tensor.matmul(
                            dst, lhsT=ones, rhs=wfr[:, g0 - F_LO : g0 - F_LO + gw],
                            start=False, stop=True,
                        )

                # V: d2 = (psum + xsq) + wsq_bc on VectorE into one SBUF tile
                # (two clean 1024-wide STTs), then one ScalarE sqrt pass.
                d2 = dpool.tile([M_TILE, V + G], f32, name="d2")
                for t, l0, c0, w in _tile_splits(0, V + G):
                    nc.vector.scalar_tensor_tensor(
                        d2[:, c0 : c0 + w], ps[t][:, l0 : l0 + w],
                        xsqb[:, mb], wbc[:, c0 : c0 + w], ADD, ADD,
                    )
                if V:
                    nc.scalar.activation(
                        ot[:, :V], d2[:, :V], SQRT, bias=0.0, scale=1.0
                    )

                # F: fused sqrt(psum + xsq) straight out of PSUM.
                for t, l0, c0, w in _tile_splits(F_LO, N):
                    nc.scalar.activation(
                        ot[:, c0 : c0 + w], ps[t][:, l0 : l0 + w], SQRT,
                        bias=xsqb[:, mb], scale=1.0,
                    )

                # Alternate stores between the SP HWDGE queue (Sync engine)
                # and the Pool SWDGE path (GpSimd is otherwise idle); Scalar
                # and Vector are at their busy ceilings.
                eng = nc.gpsimd if (m % 2 == 1 and cfg["store_swdge"]) else nc.sync
                eng.dma_start(out[ms, :], ot)

    nc.finalize()
    return nc


def _quad_fit(x, w, xsq, wsq):
    """Sampled-range quadratic minimax fit of sqrt on the d2 range.

    Returns (c2, r1, r2) with sqrt(y) ~= c2*(y-r1)*(y-r2) on the range."""
    rng = np.random.default_rng(12345)
    rows = rng.choice(x.shape[0], 768, replace=False)
    cross = x[rows].astype(np.float32) @ (-2.0 * w.astype(np.float32)).T
    d2 = cross + wsq[None, :].astype(np.float32) + xsq[rows, None].astype(
        np.float32
    )
    smin, smax = float(d2.min()), float(d2.max())
    span = smax - smin
    lo, hi = max(smin - 0.12 * span, 1e-3), smax + 0.12 * span
    yy = np.polynomial.chebyshev.chebpts1(512) * (hi - lo) / 2 + (lo + hi) / 2
    cf = np.polyfit(yy, np.sqrt(yy), 2, w=1.0 / np.sqrt(yy))
    roots = np.roots(cf)
    assert np.isreal(roots).all(), (cf, roots)
    r1, r2 = sorted(roots.real)
    return float(cf[0]), float(r1), float(r2)


def _prep_inputs(x, weights):
    import ml_dtypes

    cfg = _cfg()
    F = cfg["f_cols"]
    x = np.ascontiguousarray(np.asarray(x, dtype=np.float32))
    w = np.ascontiguousarray(np.asarray(weights, dtype=np.float32))
    assert x.shape == (BATCH, D), x.shape
    assert w.shape == (N, D), w.shape

    xsq = np.einsum("bd,bd->b", x, x)
    wsq = np.einsum("nd,nd->n", w, w)
    c2, r1, r2 = _quad_fit(x, w, xsq, wsq)

    fp8 = ml_dtypes.float8_e4m3
    xq = x.astype(fp8)  # [B, 256]
    wq = (-2.0 * w).astype(fp8)  # [N, 256]
    # DoubleRow packing: [p, t, cols] with contraction row = 128*t + p.
    wt8 = np.ascontiguousarray(wq.reshape(N, 2, 128).transpose(2, 1, 0))

    # Full-rank fold operand: 128 f16 rows summing to wsq for the tail F
    # columns (residual folded into the last row).
    wtail = wsq[N - F :] if F else wsq[:1]
    wfr = np.tile((wtail / 128).astype(np.float16), (128, 1))
    resid = wtail - wfr.astype(np.float32).sum(axis=0)
    wfr[127] = (wfr[127].astype(np.float32) + resid).astype(np.float16)
    wbc = np.tile(wsq.astype(np.float16), (128, 1))  # [128, N] broadcast

    qc2 = np.full((M_TILE, 1), c2, np.float32)
    nr1 = np.full((M_TILE, 1), -r1, np.float32)
    nr2 = np.full((M_TILE, 1), -r2, np.float32)

    in_maps = []
    for c in range(N_CORES):
        bs = slice(c * BS, (c + 1) * BS)
        xt8 = np.ascontiguousarray(
            xq[bs].reshape(BS, 2, 128).transpose(2, 1, 0)
        )  # [128, 2, BS]
        xsq_t = np.ascontiguousarray(
            xsq[bs].reshape(M_TILES, M_TILE).T
        )  # [128, 32]
        in_maps.append(
            {
                "xt8": xt8,
                "wt8": wt8,
                "wfr": np.ascontiguousarray(wfr),
                "wbc": np.ascontiguousarray(wbc),
                "xsqb": xsq_t,
                "xr1": np.ascontiguousarray(xsq_t - np.float32(r1)),
                "xr2": np.ascontiguousarray(xsq_t - np.float32(r2)),
                "nr1": nr1,
                "nr2": nr2,
                "qc2": qc2,
            }
        )
    return in_maps


def run(x, weights, trace=False, nc=None, **kwargs):
    from concourse.bass_utils import run_bass_kernel_spmd

    if nc is None:
        if "nc" not in _CACHE:
            _CACHE["nc"] = _build_bass()
        nc = _CACHE["nc"]
    in_maps = _prep_inputs(x, weights)
    res = run_bass_kernel_spmd(
        nc, in_maps, core_ids=list(range(N_CORES)), trace=trace, **kwargs
    )
    out = np.concatenate(
        [res.results[c]["out"].astype(np.float32) for c in range(N_CORES)],
        axis=0,
    )
    return out, res


def _get_runner():
    """Build + jit the SPMD executable once; reuse across kernel() calls."""
    if "runner" in _CACHE:
        return _CACHE["runner"]

    import jax
    import concourse.mybir as mybir
    from concourse import bass2jax
    from jax.sharding import Mesh, PartitionSpec
    from jax.experimental.shard_map import shard_map

    bass2jax.install_neuronx_cc_hook()
    if "nc" not in _CACHE:
        _CACHE["nc"] = _build_bass()
    nc = _CACHE["nc"]

    partition_name = (
        nc.partition_id_tensor.name if nc.partition_id_tensor else None
    )
    in_names, out_names, out_avals, zero_templates = [], [], [], []
    for alloc in nc.m.functions[0].allocations:
        if not isinstance(alloc, mybir.MemoryLocationSet):
            continue
        name = alloc.memorylocations[0].name
        if alloc.kind == "ExternalInput":
            if name != partition_name:
                in_names.append(name)
        elif alloc.kind == "ExternalOutput":
            out_names.append(name)
            shape = tuple(alloc.tensor_shape)
            dtype = mybir.dt.np(alloc.dtype)
            out_avals.append(jax.core.ShapedArray(shape, dtype))
            zero_templates.append((shape, dtype))
    n_params = len(in_names)
    n_outs = len(out_names)
    all_names = in_names + out_names
    if partition_name is not None:
        all_names = all_names + [partition_name]
    donate = tuple(range(n_params, n_params + n_outs))

    def _body(*args):
        operands = list(args)
        if partition_name is not None:
            operands.append(bass2jax.partition_id_tensor())
        outs = bass2jax._bass_exec_p.bind(
            *operands,
            out_avals=tuple(out_avals),
            in_names=tuple(all_names),
            out_names=tuple(out_names),
            lowering_input_output_aliases=(),
            sim_require_finite=True,
            sim_require_nnan=True,
            nc=nc,
        )
        return tuple(outs)

    devices = jax.devices()[:N_CORES]
    mesh = Mesh(np.asarray(devices), ("core",))
    specs = (PartitionSpec("core"),) * (n_params + n_outs)
    sharded = jax.jit(
        shard_map(
            _body, mesh=mesh, in_specs=specs, out_specs=specs[:n_outs],
            check_rep=False,
        ),
        donate_argnums=donate,
        keep_unused=True,
    )

    def runner(in_maps):
        concat_in = [
            np.concatenate([m[name] for m in in_maps], axis=0)
            for name in in_names
        ]
        concat_zeros = [
            np.zeros((N_CORES * s[0], *s[1:]), d) for s, d in zero_templates
        ]
        out_arrs = sharded(*concat_in, *concat_zeros)
        return np.asarray(out_arrs[out_names.index("out")])

    _CACHE["runner"] = runner
    return runner


def kernel(x, weights):
    runner = _get_runner()
    in_maps = _prep_inputs(x, weights)
    out = runner(in_maps)
    return out.astype(np.float32)
